# revision 31
# baseline (speedup 1.0000x reference)
"""Trainium2 Bass kernel for nn_Cache_14413910245413 (topk_masking).

Reference computation (per batch b):
  q[b]    : (L=64, NHID=256)        from query (1, L, bsz, NHID)
  K[b]    : (N=512, L=64, NHID=256) from keys  (N, bsz, L*NHID)
  att[b,n]  = max_{i,j} dot(q[b,i], K[b,n,j])       -> (bsz, 1, N) fp32
  topk_idx  = top_k(att, 16) indices                -> (k, bsz, 1) int32

Sharding: batch across the 8 cores (4 batches per core), no collectives.

Per-core pipeline (all batches sequential, strips pipelined by Tile):
  DMA keys strips (natural layout, nj on partitions, h on free)
  -> PE transpose 128x128 chunks (fp32) -> PSUM
  -> DVE/ACT copy to SBUF as K^T tiles [h=128, nj=512]
  -> PE matmul (Q^T stationary)        -> PSUM att chunk [i=64, nj=512]
  -> DVE reduce_max over j (innermost 64) -> SBUF att_partial [64 i, n]
  -> GPSIMD partition_all_reduce (max over i)
  -> SBUF->SBUF DMA rearrange -> [4 b, 512 n]
  -> DVE max8/max_index/match_replace twice -> top-16 indices
"""

import os
from contextlib import ExitStack

import numpy as np

import concourse.bass as bass
import concourse.bacc as bacc
import concourse.mybir as mybir
import concourse.tile as tile
from concourse.bass_utils import run_bass_kernel_spmd
from concourse.masks import make_identity
from concourse import library_config

L = 64
NHID = 256
N = 512
TOPK = 16
BSZ = 32
NCORES = 8
BL = BSZ // NCORES          # batches per core = 4
NJ = N * L                  # 32768 key rows per batch

F32 = mybir.dt.float32
# float32r runs the PE at 1 col/cycle instead of 4 (fp32). Numerics on HW are
# not bit-identical to fp32; toggle for A/B testing.
MM_DT = mybir.dt.float32r if os.environ.get("K_MM_DT", "f32") == "f32r" else F32
NEG_BIG = -3.0e38
N_STRIP = 128               # n rows per strip (on partitions)
NSTRIPS = N // N_STRIP      # 4 n-strips per batch
JB = int(os.environ.get("K_JB", "16"))  # j values per DMA block (16 -> 2MB strips)
NJB = L // JB               # 4 j-blocks per (batch, n-strip)





def build_kernel(ctx, tc, debug=False):
    nc = tc.nc
    keysc = nc.dram_tensor("keysc", [N, BL, L * NHID], F32, kind="ExternalInput")
    qt = nc.dram_tensor("qt", [BL, 2, 128, L], F32, kind="ExternalInput")
    att_out = nc.dram_tensor("att", [BL, N], F32, kind="ExternalOutput")
    idx_out = nc.dram_tensor("idx", [BL, TOPK], mybir.dt.uint32, kind="ExternalOutput")
    if debug:
        dbg_partial = nc.dram_tensor("dbg_partial", [64, BL * N], F32, kind="ExternalOutput")
        dbg_red = nc.dram_tensor("dbg_red", [64, BL * N], F32, kind="ExternalOutput")
        dbg_rhs = nc.dram_tensor("dbg_rhs", [128, 512], F32, kind="ExternalOutput")
        dbg_qt = nc.dram_tensor("dbg_qt", [128, 2, L], F32, kind="ExternalOutput")

    const_pool = ctx.enter_context(tc.tile_pool(name="const", bufs=1))
    nat_pool = ctx.enter_context(tc.tile_pool(name="nat", bufs=int(__import__("os").environ.get("K_NAT", "3"))))
    rhs_pool = ctx.enter_context(tc.tile_pool(name="rhs", bufs=int(os.environ.get("K_RHS", "8"))))
    qt_pool = ctx.enter_context(tc.tile_pool(name="qtp", bufs=2))
    accum_pool = ctx.enter_context(tc.tile_pool(name="accum", bufs=1))
    jscr_pool = ctx.enter_context(tc.tile_pool(name="jscr", bufs=2))
    jtmp_pool = ctx.enter_context(tc.tile_pool(name="jtmp", bufs=2))
    small_pool = ctx.enter_context(tc.tile_pool(name="small", bufs=2))
    dram_pool = ctx.enter_context(tc.tile_pool(name="dstage", bufs=1, space="DRAM"))
    tpsum_pool = ctx.enter_context(tc.tile_pool(name="tpsum", bufs=int(__import__("os").environ.get("K_TPSUM", "4")), space="PSUM"))
    apsum_pool = ctx.enter_context(tc.tile_pool(name="apsum", bufs=int(__import__("os").environ.get("K_APSUM", "4")), space="PSUM"))

    ident = const_pool.tile([128, 128], F32)
    make_identity(nc, ident[:])
    # partition_all_reduce lives in the gpsimd 'attn' ucode library
    nc.gpsimd.load_library(library_config.attn)

    # att_partial[i, bl*N + n] = max_j dot(q[bl,i], K[bl,n,j])
    att_partial = accum_pool.tile([64, BL * N], F32)
    att_red = accum_pool.tile([64, BL * N], F32)
    staging = dram_pool.tile([BL * N], F32)
    att_final = small_pool.tile([BL, N], F32)

    # keysc viewed as [b, n, j, h]; strip DMA is 3D with 16KB contiguous rows
    kv = keysc.rearrange("n b (j h) -> b n j h", h=NHID)

    # software-pipelined chunk stream: emit chunk k+1's transposes+copies
    # (phase A) before chunk k's matmuls+reduce (phase B) so the PE never
    # stalls on a just-finished rhs copy.
    state = {"qt_use": None, "qt_bl": -1, "nat": None, "nat_key": None, "jscr": {}}

    def emit_A(bl, s, jb, g):
        if state["qt_bl"] != bl:
            qt_sb = qt_pool.tile([128, 2, L], F32, name=f"qt_sb_{bl}")
            nc.sync.dma_start(qt_sb[:], qt.rearrange("b c p i -> b p c i")[bl])
            if MM_DT is not F32:
                # fp32r matmul inputs must be pre-rounded to fp32r (20-bit)
                qt_r = qt_pool.tile([128, 2, L], MM_DT, name=f"qt_r_{bl}")
                nc.vector.tensor_copy(qt_r[:], qt_sb[:])
                state["qt_use"] = qt_r
            else:
                state["qt_use"] = qt_sb
            state["qt_bl"] = bl
            if debug and bl == 0:
                nc.sync.dma_start(dbg_qt[:, :, :], qt_sb[:])
        if state["nat_key"] != (bl, s, jb):
            n0 = s * N_STRIP
            nat = nat_pool.tile([128, JB, NHID], F32, name=f"nat_{bl}_{s}_{jb}", tag="nat")
            nc.sync.dma_start(
                nat[:], kv[bl, n0 : n0 + N_STRIP, jb * JB : (jb + 1) * JB, :]
            )
            state["nat"] = nat
            state["nat_key"] = (bl, s, jb)
        nat = state["nat"]
        rhs_pair = []
        for c in range(2):  # h halves
            tp = tpsum_pool.tile([128, 512], F32, name=f"tp_{bl}_{s}_{jb}_{g}_{c}", tag="tp")
            for jj in range(4):
                j = g * 4 + jj
                nc.tensor.transpose(
                    tp[:, jj * 128 : (jj + 1) * 128],
                    nat[:, j, c * 128 : (c + 1) * 128],
                    ident[:],
                )
            rhs = rhs_pool.tile([128, 512], MM_DT, tag="rhs", name=f"rhs_{bl}_{s}_{jb}_{g}_{c}")
            if c == 0:
                nc.vector.tensor_copy(rhs[:], tp[:])
            else:
                nc.scalar.copy(rhs[:], tp[:])
            rhs_pair.append(rhs)
        return (state["qt_use"], rhs_pair)

    def emit_B(handles, bl, s, jb, g):
        qt_use, rhs_pair = handles
        att_psum = apsum_pool.tile([64, 512], F32, name=f"ap_{bl}_{s}_{jb}_{g}", tag="ap")
        for c in range(2):
            nc.tensor.matmul(
                att_psum[:],
                qt_use[:, c, :],
                rhs_pair[c][:],
                start=(c == 0),
                stop=(c == 1),
            )
        if (bl, s) not in state["jscr"]:
            state["jscr"][(bl, s)] = jscr_pool.tile(
                [64, L // 4, N_STRIP], F32, name=f"jscr_{bl}_{s}", tag="jscr"
            )
        jscr = state["jscr"][(bl, s)]
        # free order inside att_psum is (j 4, n 128): reduce over j
        nc.vector.reduce_max(
            jscr[:, jb * (JB // 4) + g, :],
            att_psum.rearrange("p (j n) -> p n j", j=4),
            axis=mybir.AxisListType.X,
        )
        if jb == NJB - 1 and g == (JB // 4) - 1:
            # log-tree max over the 16 j-groups -> [64, 128]
            n0 = s * N_STRIP
            t8 = jtmp_pool.tile([64, 8, N_STRIP], F32, tag="t8", name=f"t8_{bl}_{s}")
            nc.vector.tensor_max(t8[:], jscr[:, 0:8, :], jscr[:, 8:16, :])
            t4 = jtmp_pool.tile([64, 4, N_STRIP], F32, tag="t4", name=f"t4_{bl}_{s}")
            nc.vector.tensor_max(t4[:], t8[:, 0:4, :], t8[:, 4:8, :])
            t2 = jtmp_pool.tile([64, 2, N_STRIP], F32, tag="t2", name=f"t2_{bl}_{s}")
            nc.vector.tensor_max(t2[:], t4[:, 0:2, :], t4[:, 2:4, :])
            nc.vector.tensor_max(
                att_partial[:, bl * N + n0 : bl * N + n0 + N_STRIP],
                t2[:, 0, :],
                t2[:, 1, :],
            )
            if s == NSTRIPS - 1:
                # batch bl fully reduced over j: fold i (partitions) and
                # gather to att_final[bl] now, hidden under the next batch
                nc.gpsimd.partition_all_reduce(
                    att_red[:, bl * N : (bl + 1) * N],
                    att_partial[:, bl * N : (bl + 1) * N],
                    channels=64,
                    reduce_op=bass.bass_isa.ReduceOp.max,
                )
                nc.sync.dma_start(
                    staging[bl * N : (bl + 1) * N], att_red[0:1, bl * N : (bl + 1) * N]
                )
                nc.sync.dma_start(
                    att_final[bl : bl + 1, :],
                    staging[bl * N : (bl + 1) * N].rearrange("(o n) -> o n", o=1),
                )

    chunks = [
        (bl, s, jb, g)
        for bl in range(BL)
        for s in range(NSTRIPS)
        for jb in range(NJB)
        for g in range(JB // 4)
    ]
    depth = int(os.environ.get("K_PIPE", "1"))
    pending = []
    for ch in chunks:
        h = emit_A(*ch)
        pending.append((h, *ch))
        if len(pending) > depth:
            emit_B(*pending.pop(0))
    for p in pending:
        emit_B(*p)

    if debug:
        nc.sync.dma_start(dbg_partial[:, :], att_partial[:])
        nc.sync.dma_start(dbg_red[:, :], att_red[:])

    nc.sync.dma_start(att_out[:, :], att_final[:])

    # top-16 per batch = two rounds of (max8, max_index, match_replace)
    work = small_pool.tile([BL, N], F32)
    max8 = small_pool.tile([BL, 2, 8], F32)
    idx8 = small_pool.tile([BL, 2, 8], mybir.dt.uint32)
    nc.vector.max(max8[:, 0, :], att_final[:])
    nc.vector.max_index(idx8[:, 0, :], max8[:, 0, :], att_final[:])
    nc.vector.match_replace(work[:], max8[:, 0, :], att_final[:], NEG_BIG)
    nc.vector.max(max8[:, 1, :], work[:])
    nc.vector.max_index(idx8[:, 1, :], max8[:, 1, :], work[:])
    nc.sync.dma_start(idx_out[:, :], idx8.rearrange("b r k -> b (r k)"))


_NC_CACHE = {}


def _get_nc(debug=False):
    key = (MM_DT, debug)
    if key not in _NC_CACHE:
        nc = bacc.Bacc("TRN2", target_bir_lowering=False, debug=False)
        with tile.TileContext(nc) as tc:
            with ExitStack() as ctx:
                build_kernel(ctx, tc, debug=debug)
        nc.finalize()
        _NC_CACHE[key] = nc
    return _NC_CACHE[key]


def kernel(query: np.ndarray, keys: np.ndarray, _trace: bool = False):
    query = np.asarray(query, dtype=np.float32)
    keys = np.asarray(keys, dtype=np.float32)
    assert query.shape == (1, L, BSZ, NHID), query.shape
    assert keys.shape == (N, BSZ, L * NHID), keys.shape

    nc = _get_nc()
    in_maps = []
    for core in range(NCORES):
        b0 = core * BL
        # qt[bl, c, h, i] = query[0, i, b0+bl, c*128+h]
        qt_np = np.ascontiguousarray(
            query[0, :, b0 : b0 + BL, :].transpose(1, 2, 0)
        ).reshape(BL, 2, 128, L)
        keys_np = np.ascontiguousarray(keys[:, b0 : b0 + BL, :])
        in_maps.append({"keysc": keys_np, "qt": np.ascontiguousarray(qt_np)})

    res = run_bass_kernel_spmd(
        nc, in_maps, core_ids=list(range(NCORES)), trace=_trace
    )

    att = np.concatenate([r["att"] for r in res.results], axis=0)  # (32, 512)
    idx = np.concatenate([r["idx"] for r in res.results], axis=0)  # (32, 16)
    att_full = att.reshape(BSZ, 1, N).astype(np.float32)
    idx_full = idx.astype(np.int64).astype(np.int32).T.reshape(TOPK, BSZ, 1)
    if _trace:
        return (att_full, idx_full), res
    return att_full, idx_full


# revision 33
# speedup vs baseline: 1.4583x; 1.4583x over previous
"""Trainium2 Bass kernel for nn_Cache_14413910245413 (topk_masking).

Reference computation (per batch b):
  q[b]    : (L=64, NHID=256)        from query (1, L, bsz, NHID)
  K[b]    : (N=512, L=64, NHID=256) from keys  (N, bsz, L*NHID)
  att[b,n]  = max_{i,j} dot(q[b,i], K[b,n,j])       -> (bsz, 1, N) fp32
  topk_idx  = top_k(att, 16) indices                -> (k, bsz, 1) int32

Sharding: batch across the 8 cores (4 batches per core), no collectives.

Per-core pipeline (all batches sequential, strips pipelined by Tile):
  DMA keys strips (natural layout, nj on partitions, h on free)
  -> PE transpose 128x128 chunks (fp32) -> PSUM
  -> DVE/ACT copy to SBUF as K^T tiles [h=128, nj=512]
  -> PE matmul (Q^T stationary)        -> PSUM att chunk [i=64, nj=512]
  -> DVE reduce_max over j (innermost 64) -> SBUF att_partial [64 i, n]
  -> GPSIMD partition_all_reduce (max over i)
  -> SBUF->SBUF DMA rearrange -> [4 b, 512 n]
  -> DVE max8/max_index/match_replace twice -> top-16 indices
"""

import os
from contextlib import ExitStack

import numpy as np

import concourse.bass as bass
import concourse.bacc as bacc
import concourse.mybir as mybir
import concourse.tile as tile
from concourse.bass_utils import run_bass_kernel_spmd
from concourse.masks import make_identity
from concourse import library_config

L = 64
NHID = 256
N = 512
TOPK = 16
BSZ = 32
NCORES = 8
BL = BSZ // NCORES          # batches per core = 4
NJ = N * L                  # 32768 key rows per batch

F32 = mybir.dt.float32
# float32r runs the PE at 1 col/cycle instead of 4 (fp32). Numerics on HW are
# not bit-identical to fp32; toggle for A/B testing.
MM_DT = mybir.dt.float32r if os.environ.get("K_MM_DT", "f32") == "f32r" else F32
NEG_BIG = -3.0e38
N_STRIP = 128               # n rows per strip (on partitions)
NSTRIPS = N // N_STRIP      # 4 n-strips per batch
JB = int(os.environ.get("K_JB", "16"))  # j values per DMA block (16 -> 2MB strips)
NJB = L // JB               # 4 j-blocks per (batch, n-strip)





HOST_T = os.environ.get("K_HOSTT", "0") == "1"


def build_kernel(ctx, tc, debug=False):
    nc = tc.nc
    if HOST_T:
        keyst = nc.dram_tensor("keyst", [BL, 2, 128, NJ], F32, kind="ExternalInput")
    else:
        keysc = nc.dram_tensor("keysc", [N, BL, L * NHID], F32, kind="ExternalInput")
    qt = nc.dram_tensor("qt", [BL, 2, 128, L], F32, kind="ExternalInput")
    att_out = nc.dram_tensor("att", [BL, N], F32, kind="ExternalOutput")
    idx_out = nc.dram_tensor("idx", [BL, TOPK], mybir.dt.uint32, kind="ExternalOutput")
    if debug:
        dbg_partial = nc.dram_tensor("dbg_partial", [64, BL * N], F32, kind="ExternalOutput")
        dbg_red = nc.dram_tensor("dbg_red", [64, BL * N], F32, kind="ExternalOutput")
        dbg_rhs = nc.dram_tensor("dbg_rhs", [128, 512], F32, kind="ExternalOutput")
        dbg_qt = nc.dram_tensor("dbg_qt", [128, 2, L], F32, kind="ExternalOutput")

    const_pool = ctx.enter_context(tc.tile_pool(name="const", bufs=1))
    nat_pool = ctx.enter_context(tc.tile_pool(name="nat", bufs=int(__import__("os").environ.get("K_NAT", "3"))))
    rhs_pool = ctx.enter_context(tc.tile_pool(name="rhs", bufs=int(os.environ.get("K_RHS", "8"))))
    qt_pool = ctx.enter_context(tc.tile_pool(name="qtp", bufs=2))
    accum_pool = ctx.enter_context(tc.tile_pool(name="accum", bufs=1))
    jscr_pool = ctx.enter_context(tc.tile_pool(name="jscr", bufs=2))
    jtmp_pool = ctx.enter_context(tc.tile_pool(name="jtmp", bufs=2))
    small_pool = ctx.enter_context(tc.tile_pool(name="small", bufs=2))
    dram_pool = ctx.enter_context(tc.tile_pool(name="dstage", bufs=1, space="DRAM"))
    tpsum_pool = ctx.enter_context(tc.tile_pool(name="tpsum", bufs=int(__import__("os").environ.get("K_TPSUM", "4")), space="PSUM"))
    apsum_pool = ctx.enter_context(tc.tile_pool(name="apsum", bufs=int(__import__("os").environ.get("K_APSUM", "4")), space="PSUM"))

    ident = const_pool.tile([128, 128], F32)
    make_identity(nc, ident[:])
    # partition_all_reduce lives in the gpsimd 'attn' ucode library
    nc.gpsimd.load_library(library_config.attn)

    # att_partial[i, bl*N + n] = max_j dot(q[bl,i], K[bl,n,j])
    att_partial = accum_pool.tile([64, BL * N], F32)
    att_red = accum_pool.tile([64, BL * N], F32)
    staging = dram_pool.tile([BL * N], F32)
    att_final = small_pool.tile([BL, N], F32)

    # keysc viewed as [b, n, j, h]; strip DMA is 3D with 16KB contiguous rows
    kv = None if HOST_T else keysc.rearrange("n b (j h) -> b n j h", h=NHID)

    if HOST_T:
        # keys pre-transposed on host: [bl, c, h', nj] with nj = n*64+j
        # contiguous -> no PE transposes, no PSUM copies; MMs read strips
        kst_pool = ctx.enter_context(tc.tile_pool(name="kst", bufs=4))
        NJ_BLK = 4096
        for bl in range(BL):
            qt_sb = qt_pool.tile([128, 2, L], F32, name=f"qt_sb_{bl}", tag="qts")
            nc.sync.dma_start(qt_sb[:], qt.rearrange("b c p i -> b p c i")[bl])
            for s_ in range(NJ // NJ_BLK):
                strips = []
                for c in range(2):
                    kst = kst_pool.tile([128, NJ_BLK], F32, tag="kst",
                                        name=f"kst_{bl}_{s_}_{c}")
                    nc.sync.dma_start(
                        kst[:], keyst[bl, c, :, s_ * NJ_BLK : (s_ + 1) * NJ_BLK]
                    )
                    strips.append(kst)
                for k in range(NJ_BLK // 512):
                    att_psum = apsum_pool.tile([64, 512], F32, tag="ap",
                                               name=f"ap_{bl}_{s_}_{k}")
                    for c in range(2):
                        nc.tensor.matmul(
                            att_psum[:],
                            qt_sb[:, c, :],
                            strips[c][:, k * 512 : (k + 1) * 512],
                            start=(c == 0),
                            stop=(c == 1),
                        )
                    n0 = s_ * 64 + k * 8
                    nc.vector.reduce_max(
                        att_partial[:, bl * N + n0 : bl * N + n0 + 8],
                        att_psum.rearrange("p (n j) -> p n j", j=64),
                        axis=mybir.AxisListType.X,
                    )
            nc.gpsimd.partition_all_reduce(
                att_red[:, bl * N : (bl + 1) * N],
                att_partial[:, bl * N : (bl + 1) * N],
                channels=64,
                reduce_op=bass.bass_isa.ReduceOp.max,
            )
            nc.sync.dma_start(
                staging[bl * N : (bl + 1) * N], att_red[0:1, bl * N : (bl + 1) * N]
            )
            nc.sync.dma_start(
                att_final[bl : bl + 1, :],
                staging[bl * N : (bl + 1) * N].rearrange("(o n) -> o n", o=1),
            )
        emit_main = False
    else:
        emit_main = True

    # software-pipelined chunk stream: emit chunk k+1's transposes+copies
    # (phase A) before chunk k's matmuls+reduce (phase B) so the PE never
    # stalls on a just-finished rhs copy.
    state = {"qt_use": None, "qt_bl": -1, "nat": None, "nat_key": None, "jscr": {}}

    def emit_A(bl, s, jb, g):
        if state["qt_bl"] != bl:
            qt_sb = qt_pool.tile([128, 2, L], F32, name=f"qt_sb_{bl}")
            nc.sync.dma_start(qt_sb[:], qt.rearrange("b c p i -> b p c i")[bl])
            if MM_DT is not F32:
                # fp32r matmul inputs must be pre-rounded to fp32r (20-bit)
                qt_r = qt_pool.tile([128, 2, L], MM_DT, name=f"qt_r_{bl}")
                nc.vector.tensor_copy(qt_r[:], qt_sb[:])
                state["qt_use"] = qt_r
            else:
                state["qt_use"] = qt_sb
            state["qt_bl"] = bl
            if debug and bl == 0:
                nc.sync.dma_start(dbg_qt[:, :, :], qt_sb[:])
        if state["nat_key"] != (bl, s, jb):
            n0 = s * N_STRIP
            nat = nat_pool.tile([128, JB, NHID], F32, name=f"nat_{bl}_{s}_{jb}", tag="nat")
            nc.sync.dma_start(
                nat[:], kv[bl, n0 : n0 + N_STRIP, jb * JB : (jb + 1) * JB, :]
            )
            state["nat"] = nat
            state["nat_key"] = (bl, s, jb)
        nat = state["nat"]
        rhs_pair = []
        for c in range(2):  # h halves
            tp = tpsum_pool.tile([128, 512], F32, name=f"tp_{bl}_{s}_{jb}_{g}_{c}", tag="tp")
            for jj in range(4):
                j = g * 4 + jj
                nc.tensor.transpose(
                    tp[:, jj * 128 : (jj + 1) * 128],
                    nat[:, j, c * 128 : (c + 1) * 128],
                    ident[:],
                )
            rhs = rhs_pool.tile([128, 512], MM_DT, tag="rhs", name=f"rhs_{bl}_{s}_{jb}_{g}_{c}")
            if c == 0:
                nc.vector.tensor_copy(rhs[:], tp[:])
            else:
                nc.scalar.copy(rhs[:], tp[:])
            rhs_pair.append(rhs)
        return (state["qt_use"], rhs_pair)

    def emit_B(handles, bl, s, jb, g):
        qt_use, rhs_pair = handles
        att_psum = apsum_pool.tile([64, 512], F32, name=f"ap_{bl}_{s}_{jb}_{g}", tag="ap")
        for c in range(2):
            nc.tensor.matmul(
                att_psum[:],
                qt_use[:, c, :],
                rhs_pair[c][:],
                start=(c == 0),
                stop=(c == 1),
            )
        if (bl, s) not in state["jscr"]:
            state["jscr"][(bl, s)] = jscr_pool.tile(
                [64, L // 4, N_STRIP], F32, name=f"jscr_{bl}_{s}", tag="jscr"
            )
        jscr = state["jscr"][(bl, s)]
        # free order inside att_psum is (j 4, n 128): reduce over j
        nc.vector.reduce_max(
            jscr[:, jb * (JB // 4) + g, :],
            att_psum.rearrange("p (j n) -> p n j", j=4),
            axis=mybir.AxisListType.X,
        )
        if jb == NJB - 1 and g == (JB // 4) - 1:
            # log-tree max over the 16 j-groups -> [64, 128]
            n0 = s * N_STRIP
            t8 = jtmp_pool.tile([64, 8, N_STRIP], F32, tag="t8", name=f"t8_{bl}_{s}")
            nc.vector.tensor_max(t8[:], jscr[:, 0:8, :], jscr[:, 8:16, :])
            t4 = jtmp_pool.tile([64, 4, N_STRIP], F32, tag="t4", name=f"t4_{bl}_{s}")
            nc.vector.tensor_max(t4[:], t8[:, 0:4, :], t8[:, 4:8, :])
            t2 = jtmp_pool.tile([64, 2, N_STRIP], F32, tag="t2", name=f"t2_{bl}_{s}")
            nc.vector.tensor_max(t2[:], t4[:, 0:2, :], t4[:, 2:4, :])
            nc.vector.tensor_max(
                att_partial[:, bl * N + n0 : bl * N + n0 + N_STRIP],
                t2[:, 0, :],
                t2[:, 1, :],
            )
            if s == NSTRIPS - 1:
                # batch bl fully reduced over j: fold i (partitions) and
                # gather to att_final[bl] now, hidden under the next batch
                nc.gpsimd.partition_all_reduce(
                    att_red[:, bl * N : (bl + 1) * N],
                    att_partial[:, bl * N : (bl + 1) * N],
                    channels=64,
                    reduce_op=bass.bass_isa.ReduceOp.max,
                )
                nc.sync.dma_start(
                    staging[bl * N : (bl + 1) * N], att_red[0:1, bl * N : (bl + 1) * N]
                )
                nc.sync.dma_start(
                    att_final[bl : bl + 1, :],
                    staging[bl * N : (bl + 1) * N].rearrange("(o n) -> o n", o=1),
                )

    chunks = [
        (bl, s, jb, g)
        for bl in range(BL)
        for s in range(NSTRIPS)
        for jb in range(NJB)
        for g in range(JB // 4)
    ]
    depth = int(os.environ.get("K_PIPE", "1"))
    pending = []
    if not emit_main:
        chunks = []
    for ch in chunks:
        h = emit_A(*ch)
        pending.append((h, *ch))
        if len(pending) > depth:
            emit_B(*pending.pop(0))
    for p in pending:
        emit_B(*p)

    if debug:
        nc.sync.dma_start(dbg_partial[:, :], att_partial[:])
        nc.sync.dma_start(dbg_red[:, :], att_red[:])

    nc.sync.dma_start(att_out[:, :], att_final[:])

    # top-16 per batch = two rounds of (max8, max_index, match_replace)
    work = small_pool.tile([BL, N], F32)
    max8 = small_pool.tile([BL, 2, 8], F32)
    idx8 = small_pool.tile([BL, 2, 8], mybir.dt.uint32)
    nc.vector.max(max8[:, 0, :], att_final[:])
    nc.vector.max_index(idx8[:, 0, :], max8[:, 0, :], att_final[:])
    nc.vector.match_replace(work[:], max8[:, 0, :], att_final[:], NEG_BIG)
    nc.vector.max(max8[:, 1, :], work[:])
    nc.vector.max_index(idx8[:, 1, :], max8[:, 1, :], work[:])
    nc.sync.dma_start(idx_out[:, :], idx8.rearrange("b r k -> b (r k)"))


_NC_CACHE = {}


def _get_nc(debug=False):
    key = (MM_DT, debug, HOST_T)
    if key not in _NC_CACHE:
        nc = bacc.Bacc("TRN2", target_bir_lowering=False, debug=False)
        with tile.TileContext(nc) as tc:
            with ExitStack() as ctx:
                build_kernel(ctx, tc, debug=debug)
        nc.finalize()
        _NC_CACHE[key] = nc
    return _NC_CACHE[key]


def kernel(query: np.ndarray, keys: np.ndarray, _trace: bool = False):
    query = np.asarray(query, dtype=np.float32)
    keys = np.asarray(keys, dtype=np.float32)
    assert query.shape == (1, L, BSZ, NHID), query.shape
    assert keys.shape == (N, BSZ, L * NHID), keys.shape

    nc = _get_nc()
    in_maps = []
    for core in range(NCORES):
        b0 = core * BL
        # qt[bl, c, h, i] = query[0, i, b0+bl, c*128+h]
        qt_np = np.ascontiguousarray(
            query[0, :, b0 : b0 + BL, :].transpose(1, 2, 0)
        ).reshape(BL, 2, 128, L)
        if HOST_T:
            kk = keys[:, b0 : b0 + BL, :].reshape(N, BL, L, 2, 128)
            krt = kk.transpose(1, 3, 4, 0, 2).reshape(BL, 2, 128, NJ)
            in_maps.append(
                {"keyst": np.ascontiguousarray(krt), "qt": np.ascontiguousarray(qt_np)}
            )
        else:
            keys_np = np.ascontiguousarray(keys[:, b0 : b0 + BL, :])
            in_maps.append({"keysc": keys_np, "qt": np.ascontiguousarray(qt_np)})

    res = run_bass_kernel_spmd(
        nc, in_maps, core_ids=list(range(NCORES)), trace=_trace
    )

    att = np.concatenate([r["att"] for r in res.results], axis=0)  # (32, 512)
    idx = np.concatenate([r["idx"] for r in res.results], axis=0)  # (32, 16)
    att_full = att.reshape(BSZ, 1, N).astype(np.float32)
    idx_full = idx.astype(np.int64).astype(np.int32).T.reshape(TOPK, BSZ, 1)
    if _trace:
        return (att_full, idx_full), res
    return att_full, idx_full
